# revision 23
# baseline (speedup 1.0000x reference)
"""Trainium2 Bass kernel for nn_Custom_RNN (LSTM-ish cell + vocab logits).

Computation (faithful to the reference, including its quirks):
    xe = emb[x]
    pre  = xe@Wxi.T + (h@Whi.T + bhi) + (c@Whc.T + bhc)
    i = f = tanh(pre)
    c_new = i * (c + tanh(xe@Wxc.T + h@Whc.T + bhc))
    o = tanh(xe@Wxo.T + (h@Who.T + bho) + (c_new@Who.T + bho))
    h_new = o * tanh(c_new)
    logits = h_new@Wy.T + by
    returns (logits, h_new, c_new)

Sharding: data-parallel over batch (256 rows/core) for the recurrent part,
then AllGather of h_new^T (bf16) and vocab-parallel logits GEMM
(6400 padded vocab rows per core).  All GEMMs run in bf16 with fp32 PSUM
accumulation; elementwise math in fp32.  Weights/activations are cast to
bf16 and pre-transposed on the host where cheap; the big Wy weight is
transposed on-device via DMA-xbar transpose (the PE needs the contraction
dim on SBUF partitions).
"""

import sys

sys.path.insert(0, "/opt/trn_rl_repo")

import numpy as np
import ml_dtypes

V, E, H, B = 50257, 1024, 1024, 2048
NCORES = 8
BS = B // NCORES  # 256 batch rows per core
MT = BS // 128  # 2 m-tiles per core
KT = H // 128  # 8 k-tiles over the hidden/embedding dim
VP = 6400  # padded vocab rows per core (8*6400 = 51200 >= 50257)
VPAD = VP * NCORES
# Wy transpose groups (device DMA-xbar): 3 x 2048 + 1 x 256
GROUPS = [(0, 2048), (2048, 2048), (4096, 2048), (6144, 256)]
# logits N-chunks (PSUM bank width): 12 x 512 + 1 x 256
CHUNKS = [(i * 512, 512) for i in range(12)] + [(6144, 256)]

BF16 = ml_dtypes.bfloat16

_STATE = {}


def _build_program(use_collective=True, phase="all"):
    import concourse.bass as bass
    import concourse.mybir as mybir
    import concourse.tile as tile
    from concourse import bacc
    from contextlib import ExitStack

    dt = mybir.dt
    nc = bacc.Bacc("TRN2", target_bir_lowering=False, debug=False,
                   num_devices=NCORES)

    # ---- DRAM I/O (all *T tensors arrive pre-transposed from the host) ----
    xeT_d = nc.dram_tensor("xeT_b", [H, BS], dt.bfloat16, kind="ExternalInput")
    hT_d = nc.dram_tensor("hT_b", [H, BS], dt.bfloat16, kind="ExternalInput")
    cT_d = nc.dram_tensor("cT_b", [H, BS], dt.bfloat16, kind="ExternalInput")
    cfT_d = nc.dram_tensor("cfT_b", [H, BS], dt.float32, kind="ExternalInput")
    gw_d = {
        name: nc.dram_tensor(name + "T", [E, H], dt.bfloat16,
                             kind="ExternalInput")
        for name in ("Wxi", "Whi", "Wxc", "Whc", "Wxo", "Who")
    }
    bias_d = {
        name: nc.dram_tensor(name, [128, KT], dt.float32,
                             kind="ExternalInput")
        for name in ("bhi", "bhc", "bho")
    }
    wy_d = nc.dram_tensor("Wy_b", [VP, H], dt.bfloat16, kind="ExternalInput")
    by_d = nc.dram_tensor("by_b", [1, VP], dt.bfloat16, kind="ExternalInput")

    logits_d = nc.dram_tensor("logits", [B, VP], dt.float32,
                              kind="ExternalOutput")
    hnewT_d = nc.dram_tensor("h_newT", [H, BS], dt.float32,
                             kind="ExternalOutput")
    cnewT_d = nc.dram_tensor("c_newT", [H, BS], dt.float32,
                             kind="ExternalOutput")

    f32 = dt.float32
    bf = dt.bfloat16

    with tile.TileContext(nc) as tc:
        with ExitStack() as top:
            dram = top.enter_context(tc.tile_pool(name="dram", bufs=1,
                                                  space="DRAM"))
            const = top.enter_context(tc.tile_pool(name="const", bufs=1))

            ones = const.tile([1, 128], bf)
            nc.vector.memset(ones[:], 1.0)
            by_row = const.tile([1, VP], bf)
            nc.scalar.dma_start(by_row[:], by_d.ap())

            cc_in = dram.tile([KT * 128, BS], bf)  # h_new^T of this core
            cc_out = dram.tile(
                [NCORES * KT * 128, BS], bf,
                addr_space="Shared" if use_collective else "Local")

            # ===== Phase A: recurrent cell, computed TRANSPOSED ==========
            # Everything lives as [feature(128-part), k-tile, m] so gate
            # outputs (c_new^T / h_new^T) come out in exactly the lhsT
            # layout the logits GEMM needs -- no on-device transposes, and
            # gate biases ride along the partition axis as native ACT bias.
            with ExitStack() as sa:
              if phase in ("all", "gates"):
                gwp = sa.enter_context(tc.tile_pool(name="gw", bufs=1))
                act = sa.enter_context(tc.tile_pool(name="act", bufs=1))
                gps = sa.enter_context(tc.tile_pool(name="gpsum", bufs=8,
                                                    space="PSUM"))

                # activations (already [e, m] in DRAM) -> [e(128), k, m]
                aT = {}
                for name, d in (("xe", xeT_d), ("h", hT_d), ("c", cT_d)):
                    t = act.tile([128, KT, BS], bf, name=f"{name}T")
                    nc.sync.dma_start(
                        t[:], d.ap().rearrange("(k p) m -> p k m", p=128))
                    aT[name] = t
                cf = act.tile([128, KT, BS], f32)
                nc.sync.dma_start(
                    cf[:], cfT_d.ap().rearrange("(k p) m -> p k m", p=128))

                # biases as [128, k] f32 columns
                bvec = {}
                for name in ("bhi", "bhc", "bho"):
                    t = act.tile([128, KT], f32, name=f"{name}_c")
                    nc.sync.dma_start(t[:], bias_d[name].ap())
                    bvec[name] = t
                bias_pre = act.tile([128, KT], f32)
                nc.vector.tensor_tensor(bias_pre[:], bvec["bhi"][:],
                                        bvec["bhc"][:], mybir.AluOpType.add)
                bias_o = act.tile([128, KT], f32)
                nc.vector.tensor_tensor(bias_o[:], bvec["bho"][:],
                                        bvec["bho"][:], mybir.AluOpType.add)

                # gate weights (already [e, o] in DRAM) -> [e(128), k, o],
                # split in two half-tiles for finer load/compute overlap
                gwT = {}
                for name in ("Wxi", "Whi", "Whc", "Wxc", "Wxo", "Who"):
                    halves = []
                    for hh in range(2):
                        t = gwp.tile([128, KT // 2, H], bf,
                                     name=f"{name}T{hh}")
                        nc.sync.dma_start(
                            t[:],
                            gw_d[name].ap()[:, :].rearrange(
                                "(k p) o -> p k o", p=128)[
                                    :, hh * (KT // 2):(hh + 1) * (KT // 2)])
                        halves.append(t)
                    gwT[name] = halves

                def wsl(name, k, osl):
                    return gwT[name][k // (KT // 2)][:, k % (KT // 2), osl]

                i_g = act.tile([128, KT, BS], f32, tag="f32buf", bufs=4)
                tcg = act.tile([128, KT, BS], f32, tag="f32buf", bufs=4)
                tmp = act.tile([128, KT, BS], f32, tag="f32buf", bufs=4)
                cn = act.tile([128, KT, BS], f32, tag="f32buf", bufs=4)
                cnb = act.tile([128, KT, BS], bf, tag="bfbuf", bufs=2)

                def gate(out_t, terms, bias_t):
                    """out_t[:, ot, :] = tanh(sum_k terms + bias) per o-tile."""
                    for ot in range(KT):
                        osl = slice(ot * 128, (ot + 1) * 128)
                        ps = gps.tile([128, BS], f32, tag="ps")
                        n = len(terms) * KT
                        done = 0
                        for a_t, wname in terms:
                            for k in range(KT):
                                done += 1
                                nc.tensor.matmul(
                                    ps[:], wsl(wname, k, osl), a_t[:, k, :],
                                    start=(done == 1), stop=(done == n))
                        nc.scalar.activation(
                            out_t[:, ot, :], ps[:],
                            mybir.ActivationFunctionType.Tanh,
                            bias=bias_t[:, ot:ot + 1])

                # i (=f) and tanh(inner_c)
                gate(i_g, [(aT["xe"], "Wxi"), (aT["h"], "Whi"),
                           (aT["c"], "Whc")], bias_pre)
                gate(tcg, [(aT["xe"], "Wxc"), (aT["h"], "Whc")],
                     bvec["bhc"])

                # c_new = i * (c + tanh(inner))
                nc.vector.tensor_tensor(tmp[:], cf[:], tcg[:],
                                        mybir.AluOpType.add)
                nc.vector.tensor_tensor(cn[:], i_g[:], tmp[:],
                                        mybir.AluOpType.mult)
                nc.sync.dma_start(
                    cnewT_d.ap().rearrange("(k p) m -> p k m", p=128), cn[:])
                nc.vector.tensor_copy(cnb[:], cn[:])

                # o gate (after c_new), then h_new = o * tanh(c_new)
                og = act.tile([128, KT, BS], f32, tag="f32buf", bufs=4)
                gate(og, [(aT["xe"], "Wxo"), (aT["h"], "Who"),
                          (cnb, "Who")], bias_o)
                tch = act.tile([128, KT, BS], f32, tag="f32buf", bufs=4)
                nc.scalar.activation(tch[:], cn[:],
                                     mybir.ActivationFunctionType.Tanh)
                hn = act.tile([128, KT, BS], f32, tag="f32buf", bufs=4)
                nc.vector.tensor_tensor(hn[:], og[:], tch[:],
                                        mybir.AluOpType.mult)
                nc.sync.dma_start(
                    hnewT_d.ap().rearrange("(k p) m -> p k m", p=128), hn[:])
                hnb = act.tile([128, KT, BS], bf, tag="bfbuf", bufs=2)
                nc.vector.tensor_copy(hnb[:], hn[:])
                nc.sync.dma_start(
                    cc_in.rearrange("(k p) m -> p k m", p=128), hnb[:])

            # ================= Phase B: AllGather h_new^T =================
            if phase == "logits":
                pass
            elif use_collective:
                nc.gpsimd.collective_compute(
                    "AllGather",
                    mybir.AluOpType.bypass,
                    replica_groups=[list(range(NCORES))],
                    ins=[cc_in.opt()],
                    outs=[cc_out.opt()],
                )
            else:
                # structural stand-in for local timeline analysis only
                ccov = cc_out.rearrange("(r q) m -> r q m", r=NCORES)
                for rr in range(NCORES):
                    nc.sync.dma_start(ccov[rr], cc_in[:])

            # ================= Phase C: logits (vocab-sharded) ============
            with ExitStack() as sb:
              if phase in ("all", "logits"):
                htp = sb.enter_context(tc.tile_pool(name="ht", bufs=1))
                wyp = sb.enter_context(tc.tile_pool(name="wy", bufs=2))
                bfp = sb.enter_context(tc.tile_pool(name="bfull", bufs=1))
                stg = sb.enter_context(tc.tile_pool(name="stage", bufs=2))
                lps = sb.enter_context(tc.tile_pool(name="lpsum", bufs=6,
                                                    space="PSUM"))

                # full h_new^T: [e(128), k, core, m]
                hT = htp.tile([128, KT, NCORES, BS], bf)
                ccv = cc_out.rearrange("(r k p) m -> p r k m", r=NCORES,
                                       k=KT)
                for r in range(NCORES):
                    nc.sync.dma_start(hT[:, :, r, :], ccv[:, r])

                # bias row -> broadcast to 128 partitions via K=1 matmul
                bias_full = bfp.tile([128, VP], f32)
                for c0, cs in CHUNKS:
                    psb = lps.tile([128, 512], f32, tag="psb", bufs=2)
                    nc.tensor.matmul(psb[:, :cs], ones[:],
                                     by_row[:, c0:c0 + cs],
                                     start=True, stop=True)
                    nc.vector.tensor_copy(bias_full[:, c0:c0 + cs],
                                          psb[:, :cs])

                for g0, gs in GROUPS:
                    wyT = wyp.tile([128, KT, 2048], bf, tag="wyT")
                    for k in range(KT):
                        nc.scalar.dma_start(
                            wyT[:, k, :gs],
                            wy_d.ap()[g0:g0 + gs, k * 128:(k + 1) * 128],
                            transpose=True)
                    for sub in range(gs // 512 if gs >= 512 else 1):
                        s0 = sub * 512
                        cs = min(512, gs - s0)
                        c0 = g0 + s0
                        out_t = stg.tile([128, B // 128, 512], f32,
                                         tag="out")
                        for mt in range(B // 128):
                            r, ml = divmod(mt, MT)
                            ps = lps.tile([128, 512], f32, tag="ps", bufs=6)
                            for k in range(KT):
                                nc.tensor.matmul(
                                    ps[:, :cs],
                                    hT[:, k, r, ml * 128:(ml + 1) * 128],
                                    wyT[:, k, s0:s0 + cs],
                                    start=(k == 0), stop=(k == KT - 1))
                            nc.vector.tensor_tensor(
                                out_t[:, mt, :cs], ps[:, :cs],
                                bias_full[:, c0:c0 + cs],
                                mybir.AluOpType.add)
                        nc.sync.dma_start(
                            logits_d.ap()[:, c0:c0 + cs].rearrange(
                                "(mt p) c -> p mt c", p=128),
                            out_t[:, :, :cs])

    nc.compile()
    return nc


def _get_runner():
    """Build (once) a persistent jitted SPMD runner over the 8 cores."""
    if "runner" in _STATE:
        return _STATE["runner"]

    import jax
    import jax.numpy as jnp
    import concourse.mybir as mybir
    from jax.sharding import Mesh, PartitionSpec, NamedSharding
    from jax.experimental.shard_map import shard_map
    from concourse import bass2jax

    nc = _build_program()
    bass2jax.install_neuronx_cc_hook()

    partition_name = (nc.partition_id_tensor.name
                      if nc.partition_id_tensor else None)
    in_names, out_names, out_avals = [], [], []
    for alloc in nc.m.functions[0].allocations:
        if not isinstance(alloc, mybir.MemoryLocationSet):
            continue
        name = alloc.memorylocations[0].name
        if alloc.kind == "ExternalInput":
            if name != partition_name:
                in_names.append(name)
        elif alloc.kind == "ExternalOutput":
            out_names.append(name)
            out_avals.append(jax.core.ShapedArray(
                tuple(alloc.tensor_shape), mybir.dt.np(alloc.dtype)))

    n_params = len(in_names)
    n_outs = len(out_avals)
    all_in_names = list(in_names) + list(out_names)
    if partition_name is not None:
        all_in_names.append(partition_name)

    # which inputs are identical on all cores (replicated)?
    REPLICATED = {"WxiT", "WhiT", "WxcT", "WhcT", "WxoT", "WhoT",
                  "bhi", "bhc", "bho"}

    def _body(*args):
        operands = list(args)
        if partition_name is not None:
            operands.append(bass2jax.partition_id_tensor())
        outs = bass2jax._bass_exec_p.bind(
            *operands,
            out_avals=tuple(out_avals),
            in_names=tuple(all_in_names),
            out_names=tuple(out_names),
            lowering_input_output_aliases=(),
            sim_require_finite=True,
            sim_require_nnan=True,
            nc=nc,
        )
        return tuple(outs)

    devices = jax.devices()[:NCORES]
    mesh = Mesh(np.asarray(devices), ("core",))
    shard = NamedSharding(mesh, PartitionSpec("core"))
    repl = NamedSharding(mesh, PartitionSpec())

    in_specs = tuple(
        PartitionSpec() if n in REPLICATED else PartitionSpec("core")
        for n in in_names) + (PartitionSpec("core"),) * n_outs
    out_specs = (PartitionSpec("core"),) * n_outs

    # No donation: the kernel writes every output element, so the custom
    # call can allocate outputs itself and we can reuse one set of zero
    # buffers for every invocation (keeps the hot path free of 400MB
    # zero-fills).
    sharded = jax.jit(
        shard_map(_body, mesh=mesh, in_specs=in_specs, out_specs=out_specs,
                  check_rep=False),
        keep_unused=True)

    zero_fn = jax.jit(
        lambda: tuple(
            jnp.zeros((NCORES * a.shape[0], *a.shape[1:]), a.dtype)
            for a in out_avals),
        out_shardings=tuple(shard for _ in out_avals))

    def put(name, arr):
        return jax.device_put(
            np.asarray(arr), repl if name in REPLICATED else shard)

    def make_repeat(n):
        """jit that runs the NEFF n times back-to-back on-device.  Each
        iteration's outputs feed the next iteration's output-seed operands,
        creating a data chain that cannot be DCE'd or CSE'd."""

        def _body_n(*args):
            ins, carry = args[:n_params], args[n_params:]
            for _ in range(n):
                carry = _body(*ins, *carry)
            return carry

        return jax.jit(
            shard_map(_body_n, mesh=mesh, in_specs=in_specs,
                      out_specs=out_specs, check_rep=False),
            keep_unused=True)

    runner = {
        "in_names": in_names,
        "out_names": out_names,
        "sharded": sharded,
        "zero_fn": zero_fn,
        "put": put,
        "make_repeat": make_repeat,
    }
    _STATE["runner"] = runner
    return runner


def _t8(a, dtype=None):
    """[B, H] -> per-core transposed shards stacked: [8*H, BS]."""
    at = a.astype(dtype if dtype is not None else BF16).T  # [H, B]
    return np.ascontiguousarray(
        at.reshape(H, NCORES, BS).transpose(1, 0, 2)).reshape(NCORES * H, BS)


def _prep_inputs(x, h, c, emb, Wxi, Whi, bhi, Wxc, Whc, bhc, Wxo, Who, bho,
                 Wy, by):
    """Host-side shard/layout prep. Returns {name: global array}."""
    x = np.asarray(x)
    emb = np.asarray(emb, dtype=np.float32)
    xe = emb[x]  # [B, E] gather
    h = np.asarray(h, dtype=np.float32)
    c = np.asarray(c, dtype=np.float32)

    wy_pad = np.zeros((VPAD, H), dtype=BF16)
    wy_pad[:V] = np.asarray(Wy, dtype=np.float32).astype(BF16)
    by_pad = np.zeros((NCORES, VP), dtype=BF16)
    by_pad.reshape(-1)[:V] = np.asarray(by, dtype=np.float32).astype(BF16)

    def wT(a):
        return np.ascontiguousarray(
            np.asarray(a, np.float32).astype(BF16).T)

    def bcol(b):
        return np.ascontiguousarray(
            np.asarray(b, np.float32).reshape(KT, 128).T)

    feed = {
        "xeT_b": _t8(xe),
        "hT_b": _t8(h),
        "cT_b": _t8(c),
        "cfT_b": _t8(c, np.float32),
        "Wy_b": wy_pad,
        "by_b": by_pad,
        "WxiT": wT(Wxi), "WhiT": wT(Whi), "WxcT": wT(Wxc),
        "WhcT": wT(Whc), "WxoT": wT(Wxo), "WhoT": wT(Who),
        "bhi": bcol(bhi),
        "bhc": bcol(bhc),
        "bho": bcol(bho),
    }
    return feed


def _run_device(feed_dev):
    r = _get_runner()
    if "zeros" not in _STATE:
        import jax
        _STATE["zeros"] = jax.block_until_ready(r["zero_fn"]())
    args = [feed_dev[n] for n in r["in_names"]] + list(_STATE["zeros"])
    outs = r["sharded"](*args)
    return outs


def kernel(x, h, c, emb, Wxi, Whi, bhi, Wxc, Whc, bhc, Wxo, Who, bho, Wy, by):
    r = _get_runner()
    feed = _prep_inputs(x, h, c, emb, Wxi, Whi, bhi, Wxc, Whc, bhc,
                        Wxo, Who, bho, Wy, by)
    feed_dev = {n: r["put"](n, feed[n]) for n in r["in_names"]}
    outs = _run_device(feed_dev)
    by_name = dict(zip(r["out_names"], outs))

    logits_g = np.asarray(by_name["logits"])  # [8*2048, 6400]
    logits = np.concatenate(
        [logits_g[i * B:(i + 1) * B] for i in range(NCORES)],
        axis=1)[:, :V]
    hT_g = np.asarray(by_name["h_newT"])  # [8*1024, 256]
    cT_g = np.asarray(by_name["c_newT"])
    h_new = np.concatenate(
        [hT_g[r * H:(r + 1) * H].T for r in range(NCORES)], axis=0)
    c_new = np.concatenate(
        [cT_g[r * H:(r + 1) * H].T for r in range(NCORES)], axis=0)
    return logits.astype(np.float32), h_new, c_new
